# revision 17
# baseline (speedup 1.0000x reference)
"""Trainium2 Bass kernel for an 8-head MultiHeadAttention (b=8, s=1024, d=512).

Sharding: pure data-parallel over batch -- each of the 8 NeuronCores runs the
full attention for one batch element. No collectives.

v3 (matmul operands bf16, accumulate fp32). DMA reality on this part:
HWDGE queues move ~30GB/s on 512B packets but ~130GB/s on 2KB rows, the
gpsimd SWDGE aggregates descriptors (~135GB/s), and the DMA-transpose xbar
crawls at ~24GB/s -- so transposes stay on the PE and the loads are shaped
for packet size:
  sync  q: x as two full-row halves (2KB packets), then m7+wo f32.
  scalar q: wq full-row f32.
  SWDGE : wk/wv as f32->bf16 casts, mask strips 0-6 as bf16 casts (1/4 the
          f32 bytes), all aggregation-friendly.

  Q^T[hd,s] = wq^T.T @ x^T   (scale 1/8 + bias folded into the PSUM drain;
              chunk-0 q/k interleaved at j-half granularity so matmuls start
              as soon as the first x half is transposed)
  K^T[hd,s] = wk^T.T @ x^T
  V[s,hd]   = x^T.T @ wv^T   (+ bv via rank-1 ones matmul; ones col per head)
  S^T[k,q]  = K_h^T.T @ Q_h^T  -- head-pair concurrent via PE 64-row tiling.
  P^T       = exp(S^T) * (1-mask)^T  (exp on ACT)
  O^T_h[65,q] = V_aug.T @ P^T  (row 64 = softmax denominator via ones col)
  normalize: recip via ACT ln->exp(-x) (activation tables patched so exp/ln
             share one set => zero table switches), indicator-matmul
             broadcast, in-place DVE mul.
  out[q,d]  = O^T.T @ wo^T + bo (bo via rank-1 ones matmul)

Schedule: kc-granular software pipeline. A gated warm-up burst holds the PE
HAM clock-gate open just before the first projection. PV pumping cascades:
during S(p) the scheduler drains pair p-1 at up to 3 steps/slot and starts
pair p lagged, so the tail only carries ~half of pair 3's PV plus the final
projections (with keep-warm matmuls between output groups).
PSUM banks: psc [128,1024]x3 + ppv [128,512]x2 = 8.
"""

import numpy as np

P = 128
S = 1024  # sequence length
D = 512  # d_model
H = 8  # heads
DK = 64  # head dim
CH = D // P  # 4 hd/dmodel chunks
ST = S // P  # 8 seq tiles
NCORES = 8

# mask-mul strips handled by gpsimd for pairs 1-3 (by kc index)
GP_MUL_KC = (6, 7)

_CACHE = {}


def _patch_act_tables():
    """Force every activation function to resolve to the combined
    natural_log_exp_and_others set so exp and ln share one table load."""
    import concourse.hw_specs as hw_specs
    import concourse.bacc as bacc_mod

    if getattr(hw_specs, "_mha_patched", False):
        return
    _orig = hw_specs.get_activation_tables

    def _patched(arch):
        t = _orig(arch)
        return {
            name: (fns if name == "natural_log_exp_and_others" else set())
            for name, fns in t.items()
        }

    hw_specs.get_activation_tables = _patched
    hw_specs._mha_patched = True
    if hasattr(bacc_mod, "get_activation_tables"):
        bacc_mod.get_activation_tables = _patched


def _build():
    _patch_act_tables()
    import concourse.bacc as bacc
    import concourse.mybir as mybir
    import concourse.tile as tile
    from concourse.masks import make_identity

    f32 = mybir.dt.float32
    mmdt = mybir.dt.bfloat16
    AF = mybir.ActivationFunctionType
    OP = mybir.AluOpType

    nc = bacc.Bacc(None, target_bir_lowering=False, debug=False)

    x_t = nc.dram_tensor("x", [S, D], f32, kind="ExternalInput")
    mask_t = nc.dram_tensor("mask", [S, S], f32, kind="ExternalInput")
    wq_t = nc.dram_tensor("wq", [D, D], f32, kind="ExternalInput")
    wk_t = nc.dram_tensor("wk", [D, D], f32, kind="ExternalInput")
    wv_t = nc.dram_tensor("wv", [D, D], f32, kind="ExternalInput")
    wo_t = nc.dram_tensor("wo", [D, D], f32, kind="ExternalInput")
    bq_t = nc.dram_tensor("bq", [D], f32, kind="ExternalInput")
    bk_t = nc.dram_tensor("bk", [D], f32, kind="ExternalInput")
    bv_t = nc.dram_tensor("bv", [D], f32, kind="ExternalInput")
    bo_t = nc.dram_tensor("bo", [D], f32, kind="ExternalInput")
    out_t = nc.dram_tensor("out", [S, D], f32, kind="ExternalOutput")

    with tile.TileContext(nc) as tc:
        with (
            tc.tile_pool(name="persist", bufs=1) as pp,
            tc.tile_pool(name="stage", bufs=1) as stage,
            tc.tile_pool(name="mstage", bufs=5) as mstage,
            tc.tile_pool(name="ptp", bufs=4) as ptp,
            tc.tile_pool(name="nrm", bufs=2) as nrm,
            tc.tile_pool(name="fin", bufs=3) as fpool,
            tc.tile_pool(name="psc", bufs=3, space="PSUM") as psc,
            tc.tile_pool(name="ppv", bufs=2, space="PSUM") as ppv,
        ):
            # ---- constants ----
            ident = pp.tile([P, P], f32, name="id", tag="id")
            make_identity(nc, ident[:])
            ident_bf = pp.tile([P, P], mmdt, name="idb", tag="idb")
            nc.vector.tensor_copy(ident_bf[:], ident[:])
            ones_f32 = pp.tile([P, P], f32, name="ones_f32", tag="ones_f32")
            nc.vector.memset(ones_f32[:], 1.0)
            ones_sb = pp.tile([1, P], mmdt, name="ones", tag="ones")
            nc.vector.tensor_copy(ones_sb[:], ones_f32[0:1, :])
            ones512 = pp.tile([1, 512], mmdt, name="ones512", tag="ones512")
            nc.vector.memset(ones512[:], 1.0)
            # hoist the (single) activation table load to t=0
            warmact = pp.tile([1, 2], f32, name="warmact", tag="warmact")
            nc.scalar.activation(warmact[:], ones_f32[0:1, 0:2], AF.Identity)
            # indicator for the recip broadcast
            e4 = pp.tile([P, 2 * P], mmdt, name="e4", tag="e4")
            nc.vector.memset(e4[:], 0.0)
            for j in range(2):
                nc.vector.memset(
                    e4[32 * 2 * j : 32 * 2 * j + 1, j * P : j * P + 64], 1.0
                )
                nc.vector.memset(
                    e4[32 * (2 * j + 1) : 32 * (2 * j + 1) + 1,
                       j * P + 64 : (j + 1) * P], 1.0
                )

            bq_sb = pp.tile([P, CH], f32, name="bq", tag="bq")
            bk_sb = pp.tile([P, CH], f32, name="bk", tag="bk")
            qbias_sb = pp.tile([P, CH], f32, name="qbias", tag="qbias")
            bv_row = pp.tile([1, D], f32, name="bvr", tag="bvr")
            bo_row = pp.tile([1, D], f32, name="bor", tag="bor")
            bv_bf = pp.tile([1, D], mmdt, name="bvb", tag="bvb")
            bo_bf = pp.tile([1, D], mmdt, name="bob", tag="bob")

            # ---- input DMAs ----
            # sync (q1): x in two full-row halves (2KB packets), then mask
            # strip 7 (f32) and wo (f32).  x_sb borrows a ptp ring slot
            # (same byte size as a pt tile; dead before pair 1 needs it).
            x_sb = ptp.tile([P, ST, D], f32, name="xsb", tag="pt")
            for h in range(2):
                nc.sync.dma_start(
                    out=x_sb[:, 4 * h : 4 * h + 4, :],
                    in_=x_t[:].rearrange("(i p) d -> p i d", p=P)[
                        :, 4 * h : 4 * h + 4, :
                    ],
                )
            # scalar (q10): wq full-row f32 (wo later reuses this slot)
            wq_sb = stage.tile([P, CH, D], f32, name="wqsb", tag="wqwo")
            nc.scalar.dma_start(
                out=wq_sb[:], in_=wq_t[:].rearrange("(r p) d -> p r d", p=P)
            )
            # small bias loads (tiny DMAs ride the instruction queue)
            nc.sync.dma_start(out=bq_sb[:], in_=bq_t[:].rearrange("(c p) -> p c", p=P))
            nc.sync.dma_start(out=bk_sb[:], in_=bk_t[:].rearrange("(c p) -> p c", p=P))
            nc.sync.dma_start(out=bv_row[:], in_=bv_t[None, :])
            nc.sync.dma_start(out=bo_row[:], in_=bo_t[None, :])
            nc.vector.tensor_scalar_mul(qbias_sb[:], bq_sb[:], 0.125)
            nc.vector.tensor_copy(bv_bf[:], bv_row[:])
            nc.vector.tensor_copy(bo_bf[:], bo_row[:])

            # SWDGE (gpsimd, aggregating): wk/wv as bf16 casts, mask strips
            # 0-6 as bf16 casts (1/4 the f32 bytes)
            wk_sb = stage.tile([P, CH, D], mmdt, name="wksb", tag="wksb")
            wv_sb = stage.tile([P, CH, D], mmdt, name="wvsb", tag="wvsb")
            nc.gpsimd.dma_start(
                out=wk_sb[:], in_=wk_t[:].rearrange("(r p) d -> p r d", p=P)
            )
            msk = {}

            def msk_dma(kc, eng, dt, tag="msk", bufs=None):
                m = mstage.tile([P, ST, P], dt, name="msk", tag=tag,
                                **({} if bufs is None else {"bufs": bufs}))
                eng.dma_start(
                    out=m[:],
                    in_=mask_t[:, kc * P : (kc + 1) * P].rearrange(
                        "(i p) k -> p i k", p=P
                    ),
                )
                msk[kc] = m

            for kc in range(3):
                msk_dma(kc, nc.gpsimd, mmdt)
            nc.gpsimd.dma_start(
                out=wv_sb[:], in_=wv_t[:].rearrange("(r p) d -> p r d", p=P)
            )
            for kc in range(3, 7):
                msk_dma(kc, nc.gpsimd, mmdt)
            # strip 7 f32 on sync after x; wo f32 on sync after that
            # (reuses wq's staging slot -- wq is dead after its transposes)
            msk_dma(7, nc.sync, f32, tag="msk7", bufs=1)
            wo_sb = stage.tile([P, CH, D], f32, name="wosb", tag="wqwo")
            nc.sync.dma_start(
                out=wo_sb[:], in_=wo_t[:].rearrange("(r p) d -> p r d", p=P)
            )

            # ---- gated PE warm-up ----
            gate2 = pp.tile([1, P], mmdt, name="gate2", tag="gate2")
            nc.vector.tensor_copy(gate2[:], msk[2][0:1, 0, 0:P])

            def warm(n=2, lhs=None):
                jp = ppv.tile([P, 512], f32, name="pv", tag="pv")
                lhs = ones_sb if lhs is None else lhs
                for _ in range(n):
                    nc.tensor.matmul(
                        jp[0:64, 0:512], lhs[:, 0:64], ones512[:],
                        start=True, stop=True,
                    )

            # ---- transposes: x, wq, wk (PE + drain copies) ----
            xT = pp.tile([P, CH, S], mmdt, name="xT", tag="xT")

            def xpose_x_half(h):
                # x half h covers i-tiles 4h..4h+3 for all 4 chunks
                for c in range(CH):
                        # full-size slot from the shared ring; use half
                    ps = psc.tile([P, S], f32, name="ps", tag="ps")
                    ps = ps[:, 0 : S // 2]
                    for i in range(4):
                        nc.tensor.transpose(
                            ps[:, i * P : (i + 1) * P],
                            x_sb[:, 4 * h + i, c * P : (c + 1) * P],
                            ident[:],
                        )
                    if c % 2 == 0:
                        nc.scalar.activation(
                            xT[:, c, h * 512 : (h + 1) * 512], ps[:], AF.Identity
                        )
                    else:
                        nc.vector.tensor_copy(
                            xT[:, c, h * 512 : (h + 1) * 512], ps[:]
                        )

            wT = {}
            for n in ("wq", "wk", "wv", "wo"):
                wT[n] = pp.tile([P, CH, D], mmdt, name="T", tag=f"T{n}")

            def build_wT_chunk(name, src, c, drain):
                ps = ppv.tile([P, 512], src.dtype, name="pv", tag="pv")
                idd = ident if src.dtype == f32 else ident_bf
                for rr in range(CH):
                    nc.tensor.transpose(
                        ps[:, rr * P : (rr + 1) * P], src[:, rr, c * P : (c + 1) * P],
                        idd[:],
                    )
                if drain == "act":
                    nc.scalar.activation(wT[name][:, c, :], ps[:], AF.Identity)
                else:
                    nc.vector.tensor_copy(wT[name][:, c, :], ps[:])

            xpose_x_half(0)
            for c in range(CH):
                build_wT_chunk("wq", wq_sb, c, "act" if c % 2 == 0 else "dve")
            for c in range(CH):
                build_wT_chunk("wk", wk_sb, c, "act" if c % 2 else "dve")
            # warm-up burst bridges the gap until the second x half lands,
            # so the first projection matmuls run at full clock
            warm(8, lhs=gate2)
            xpose_x_half(1)

            # ---- projections Q^T, K^T (chunk 0 interleaved by j-half) ----
            qT = pp.tile([P, CH, S], mmdt, name="qT", tag="qT")
            kT = pp.tile([P, CH, S], mmdt, name="kT", tag="kT")
            omT = pp.tile([P, ST, S], mmdt, name="omT", tag="omT")

            def build_om(kc):
                ps = psc.tile([P, S], msk[kc].dtype, name="ps", tag="ps")
                idd = ident if msk[kc].dtype == f32 else ident_bf
                for qi in range(ST):
                    nc.tensor.transpose(
                        ps[:, qi * P : (qi + 1) * P], msk[kc][:, qi, :], idd[:]
                    )
                nc.vector.tensor_scalar(
                    omT[:, kc, :], ps[:], -1.0, 1.0, op0=OP.mult, op1=OP.add
                )

            def proj_drain(c, dst, bias, scale, on_act, ps):
                if on_act:
                    nc.scalar.activation(
                        dst[:, c, :], ps[:], AF.Identity,
                        bias=bias[:, c : c + 1], scale=scale,
                    )
                else:
                    nc.vector.tensor_scalar(
                        dst[:, c, :], ps[:], scale, bias[:, c : c + 1],
                        op0=OP.mult, op1=OP.add,
                    )

            def proj_qk_dst(c, dst, wname, bias, scale, on_act):
                ps = psc.tile([P, S], f32, name="ps", tag="ps")
                for j in range(2):
                    for rr in range(CH):
                        nc.tensor.matmul(
                            ps[:, j * 512 : (j + 1) * 512],
                            wT[wname][:, rr, c * P : (c + 1) * P],
                            xT[:, rr, j * 512 : (j + 1) * 512],
                            start=(rr == 0),
                            stop=(rr == CH - 1),
                        )
                proj_drain(c, dst, bias, scale, on_act, ps)

            psq = psc.tile([P, S], f32, name="ps", tag="ps")
            psk = psc.tile([P, S], f32, name="ps", tag="ps")

            def proj0_half(ps, wname, j):
                for rr in range(CH):
                    nc.tensor.matmul(
                        ps[:, j * 512 : (j + 1) * 512],
                        wT[wname][:, rr, 0:P],
                        xT[:, rr, j * 512 : (j + 1) * 512],
                        start=(rr == 0),
                        stop=(rr == CH - 1),
                    )

            proj0_half(psq, "wq", 0)
            proj0_half(psk, "wk", 0)
            build_om(0)
            proj0_half(psq, "wq", 1)
            proj0_half(psk, "wk", 1)
            proj_drain(0, qT, qbias_sb, 0.125, True, psq)
            proj_drain(0, kT, bk_sb, 1.0, True, psk)

            # ---- persistent attention state ----
            v_sb = pp.tile([P, ST, H * 65], mmdt, name="v", tag="v")
            oT = pp.tile([P, CH, S], mmdt, name="oT", tag="oT")

            def proj_v_unit(i):
                ps = ppv.tile([P, 512], f32, name="pv", tag="pv")
                for rr in range(CH):
                    nc.tensor.matmul(
                        ps[:],
                        xT[:, rr, i * P : (i + 1) * P],
                        wT["wv"][:, rr, :],
                        start=(rr == 0),
                        stop=False,
                    )
                nc.tensor.matmul(
                    ps[:], ones_sb[:, 0:P], bv_bf[:], start=False, stop=True
                )
                nc.vector.tensor_copy(
                    v_sb[:, i, :].rearrange("p (h e) -> p h e", e=65)[:, :, 0:64],
                    ps[:].rearrange("p (h e) -> p h e", e=64),
                )

            f_acc = pp.tile([P, ST, 512], mmdt, name="facc", tag="facc")
            finals = []

            def partial_qt(qt):
                if qt % 2 == 0:
                    finals.append(psc.tile([P, S], f32, name="ps", tag="ps"))
                half = finals[-1][:, (qt % 2) * 512 : (qt % 2) * 512 + 512]
                for cc in range(CH - 1):
                    nc.tensor.matmul(
                        half,
                        oT[:, cc, qt * P : (qt + 1) * P],
                        wT["wo"][:, cc, :],
                        start=(cc == 0),
                        stop=False,
                    )
                nc.tensor.matmul(
                    half, ones_sb[:, 0:P], bo_bf[:], start=False, stop=True
                )
                nc.vector.tensor_copy(f_acc[:, qt, :], half)

            # ---- pipelined attention ----
            from collections import deque

            pts = {}
            pvs = {}
            dns = {}
            pvq = {}

            def scores_unit(p, kc):
                c = p
                ptA, ptB = pts[2 * p], pts[2 * p + 1]
                kA = kT[0:64, c, kc * P : (kc + 1) * P]
                kB = kT[64:128, c, kc * P : (kc + 1) * P]
                psA = psc.tile([P, S], f32, name="ps", tag="ps")
                psB = psc.tile([P, S], f32, name="ps", tag="ps")
                for j in range(2):
                    nc.tensor.matmul(
                        psA[:, j * 512 : (j + 1) * 512],
                        kA, qT[0:64, c, j * 512 : (j + 1) * 512],
                        start=True, stop=True,
                    )
                    nc.tensor.matmul(
                        psB[:, j * 512 : (j + 1) * 512],
                        kB, qT[64:128, c, j * 512 : (j + 1) * 512],
                        start=True, stop=True,
                    )
                nc.scalar.activation(ptA[:, kc, :], psA[:], AF.Exp)
                nc.scalar.activation(ptB[:, kc, :], psB[:], AF.Exp)
                if p == 0:
                    # S(0): DVE also builds om strips; split the pair's muls
                    nc.gpsimd.tensor_mul(ptA[:, kc, :], ptA[:, kc, :], omT[:, kc, :])
                    nc.vector.tensor_mul(ptB[:, kc, :], ptB[:, kc, :], omT[:, kc, :])
                else:
                    eng = nc.gpsimd if kc in GP_MUL_KC else nc.vector
                    eng.tensor_mul(ptA[:, kc, :], ptA[:, kc, :], omT[:, kc, :])
                    eng.tensor_mul(ptB[:, kc, :], ptB[:, kc, :], omT[:, kc, :])

            def pv_start(p, j):
                pvA = ppv.tile([P, 512], f32, name="pv", tag="pv")
                pvB = ppv.tile([P, 512], f32, name="pv", tag="pv")
                pvs[(p, j)] = (pvA, pvB)

            def pv_steps(p, j, kcs):
                hA, hB = 2 * p, 2 * p + 1
                ptA, ptB = pts[hA], pts[hB]
                vA = v_sb[:].rearrange("p i (h e) -> p i h e", e=65)[:, :, hA, :]
                vB = v_sb[:].rearrange("p i (h e) -> p i h e", e=65)[:, :, hB, :]
                jsl = slice(j * 512, (j + 1) * 512)
                pvA, pvB = pvs[(p, j)]
                for kc in kcs:
                    st = kc == 0
                    sp = kc == ST - 1
                    nc.tensor.matmul(
                        pvA[0:65, :], vA[:, kc, :], ptA[:, kc, jsl],
                        start=st, stop=sp,
                    )
                    nc.tensor.matmul(
                        pvB[0:65, :], vB[:, kc, :], ptB[:, kc, jsl],
                        start=st, stop=sp,
                    )

            def pv_drain(p, j):
                c = p
                hA, hB = 2 * p, 2 * p + 1
                jsl = slice(j * 512, (j + 1) * 512)
                pvA, pvB = pvs.pop((p, j))
                dn = dns[p]
                for idx, (h, pv) in enumerate(((hA, pvA), (hB, pvB))):
                    off = 64 * (h % 2)
                    slot = 32 * (2 * j + idx)
                    nc.vector.tensor_copy(oT[off : off + 64, c, jsl], pv[0:64, :])
                    nc.vector.tensor_copy(dn[slot : slot + 1, :], pv[64:65, :])

            def pv_norm(p, then=None):
                c = p
                hA, hB = 2 * p, 2 * p + 1
                dn = dns.pop(p)
                lnd = nrm.tile([P, 512], f32, name="lnd", tag="lnd")
                nc.scalar.activation(lnd[:], dn[:], AF.Ln)
                rb4 = nrm.tile([P, 512], mmdt, name="rb4", tag="rb4")
                nc.scalar.activation(rb4[:], lnd[:], AF.Exp, scale=-1.0)
                for j in range(2):
                    jsl = slice(j * 512, (j + 1) * 512)
                    bp = psc.tile([P, S], f32, name="ps", tag="ps")
                    nc.tensor.matmul(
                        bp[:, 0:512], e4[:, j * P : (j + 1) * P], rb4[:],
                        start=True, stop=True,
                    )
                    for idx, h in enumerate((hA, hB)):
                        off = 64 * (h % 2)
                        osl = oT[off : off + 64, c, jsl]
                        nc.vector.tensor_mul(
                            osl, osl, bp[64 * idx : 64 * idx + 64, 0:512]
                        )
                    if then is not None:
                        then(j)

            def new_pair(p):
                pts[2 * p] = ptp.tile([P, ST, S], mmdt, name="pt", tag="pt")
                pts[2 * p + 1] = ptp.tile([P, ST, S], mmdt, name="pt", tag="pt")
                dns[p] = nrm.tile([P, 512], f32, name="dn", tag="dn")
                nc.vector.memset(dns[p][:], 1.0)
                pvq[p] = deque((j, kc) for j in (0, 1) for kc in range(ST))

            # ---- PV-step scheduler ----
            started = set()
            jdone = {}

            def pump(order, avail, budget):
                for p_, limit in order:
                    q = pvq.get(p_)
                    if p_ > 0 and (p_ - 1) in pvq:
                        continue  # previous pair still owns the ppv ring
                    while q and budget > 0 and limit > 0:
                        j, kc = q[0]
                        if kc > avail.get(p_, ST):
                            break
                        q.popleft()
                        if (p_, j) not in started:
                            started.add((p_, j))
                            pv_start(p_, j)
                        pv_steps(p_, j, [kc])
                        jdone[(p_, j)] = jdone.get((p_, j), 0) + 1
                        if jdone[(p_, j)] == ST:
                            pv_drain(p_, j)
                            if jdone.get((p_, 1)) == ST:
                                del pvq[p_]
                                if p_ != H // 2 - 1:
                                    pv_norm(p_)
                                break
                        budget -= 1
                        limit -= 1

            # ---- S(0): om + scores(0) + V projection ----
            new_pair(0)
            nc.vector.tensor_copy(
                v_sb[:].rearrange("p i (h e) -> p i h e", e=65)[:, :, :, 64],
                ones_f32[:, 0 : ST * H].rearrange("p (i h) -> p i h", h=H),
            )
            # wv^T chunks built in the first S(0) slots; V units follow
            VPROJ = {4: (0, 1), 5: (2, 3), 6: (4, 5), 7: (6, 7)}
            for kc in range(ST):
                if kc >= 1:
                    build_om(kc)
                scores_unit(0, kc)
                if kc == 0:
                    proj_qk_dst(1, qT, "wq", qbias_sb, 0.125, True)
                if kc == 1:
                    proj_qk_dst(1, kT, "wk", bk_sb, 1.0, True)
                if kc < CH:
                    build_wT_chunk("wv", wv_sb, kc, "dve")
                for i in VPROJ.get(kc, ()):
                    proj_v_unit(i)

            # ---- S(1..3): cascaded pump (pair p-1 fast-drained, pair p
            # starts lagged within its own scores phase) ----
            bg = {
                (1, 3): lambda: proj_qk_dst(2, qT, "wq", qbias_sb, 0.125, False),
                (1, 6): lambda: proj_qk_dst(2, kT, "wk", bk_sb, 1.0, False),
                (2, 0): lambda: build_wT_chunk("wo", wo_sb, 0, "dve"),
                (2, 2): lambda: proj_qk_dst(3, qT, "wq", qbias_sb, 0.125, False),
                (2, 3): lambda: build_wT_chunk("wo", wo_sb, 1, "act"),
                (2, 5): lambda: proj_qk_dst(3, kT, "wk", bk_sb, 1.0, False),
                (2, 6): lambda: build_wT_chunk("wo", wo_sb, 2, "dve"),
                (3, 0): lambda: build_wT_chunk("wo", wo_sb, 3, "act"),
            }

            for p in (1, 2, 3):
                new_pair(p)
                for kc in range(ST):
                    scores_unit(p, kc)
                    if (p, kc) in bg:
                        bg[(p, kc)]()
                    pump([(p - 1, 3), (p, 2)], {p - 1: ST, p: kc - 1}, 3)

            # tail: remainder of pair-3 PV interleaved with the partial
            # output projections
            for qtp in range(4):
                pump([(3, 4)], {3: ST}, 4)
                partial_qt(2 * qtp)
                partial_qt(2 * qtp + 1)
            while pvq:
                pump([(0, 8), (1, 8), (2, 8), (3, 8)], {0: ST, 1: ST, 2: ST, 3: ST}, 16)

            warm(6)

            def tail_finals(j):
                for qt in range(4 * j, 4 * j + 4):
                    if qt % 2 == 0:
                        finals.append(psc.tile([P, S], f32, name="ps", tag="ps"))
                    half = finals[-1][:, (qt % 2) * 512 : (qt % 2) * 512 + 512]
                    nc.tensor.matmul(
                        half,
                        oT[:, 3, qt * P : (qt + 1) * P],
                        wT["wo"][:, 3, :],
                        start=True, stop=True,
                    )
                    ft = fpool.tile([P, 512], f32, name="fin", tag="fin")
                    nc.vector.tensor_add(ft[:], half, f_acc[:, qt, :])
                    eng = nc.sync if qt % 2 == 0 else nc.scalar
                    eng.dma_start(out=out_t[qt * P : (qt + 1) * P, :], in_=ft[:])
                warm(2)

            pv_norm(3, then=tail_finals)

    nc.compile()
    return nc


def _get_nc():
    if "nc" not in _CACHE:
        _CACHE["nc"] = _build()
    return _CACHE["nc"]


def run(inputs, trace=False, **kw):
    from concourse.bass_utils import run_bass_kernel_spmd

    nc = _get_nc()
    f = np.float32
    in_maps = [
        {
            "x": np.ascontiguousarray(inputs["inputs"][i], dtype=f),
            "mask": np.ascontiguousarray(inputs["mask"][i], dtype=f),
            "wq": np.ascontiguousarray(inputs["wq"], dtype=f),
            "wk": np.ascontiguousarray(inputs["wk"], dtype=f),
            "wv": np.ascontiguousarray(inputs["wv"], dtype=f),
            "wo": np.ascontiguousarray(inputs["wo"], dtype=f),
            "bq": np.ascontiguousarray(inputs["bq"], dtype=f),
            "bk": np.ascontiguousarray(inputs["bk"], dtype=f),
            "bv": np.ascontiguousarray(inputs["bv"], dtype=f),
            "bo": np.ascontiguousarray(inputs["bo"], dtype=f),
        }
        for i in range(NCORES)
    ]
    res = run_bass_kernel_spmd(nc, in_maps, list(range(NCORES)), trace=trace, **kw)
    out = np.stack(
        [np.asarray(res.results[i]["out"], dtype=np.float32) for i in range(NCORES)],
        axis=0,
    )
    return out, res


def kernel(**inputs):
    out, _ = run(inputs)
    return out
